# revision 10
# baseline (speedup 1.0000x reference)
"""Causal Group-Query Attention kernel for Trainium2 (8 NeuronCores, SPMD).

Problem: x[2,2048,2048] @ Wq -> q(16 heads x 128); x @ Wkv -> k,v (4 KV heads);
causal softmax attention with GQA (4 q-heads per kv-head); y @ Wc -> out.

Sharding (2 batch x 4 head-groups = 8 cores):
  core = 4*b + g handles batch b, q-heads 4g..4g+3 (= kv head g).
  Each core gets xT (x[b] transposed, [C,T]), its Wq/Wk/Wv column shards and
  Wc row shard, and produces a partial [T,C] output; host sums the 4 partials
  per batch (the "all-reduce" of the c_proj happens on host).

Per-core device pipeline (matmuls in f32r = full-rate fp32):
  1. projections: qT[d,t] per head, kT[d,t], vT -> (PE-transpose) v[t,d]
  2. attention per (head, 512-wide q strip): S^T blocks [tk=128, tq=512];
     exp on ScalarE (softmax scale fused); causal diagonal masks multiplied
     in; denominator rows via ones-column matmul accumulation; yT accumulated
     via matmul(lhsT=v_block, rhs=p_block); normalization: denom row -> PE
     outer-product broadcast -> DVE reciprocal_approx_fast -> multiply.
  3. c_proj: out[t,o] accumulated over the 4 head-dim blocks, DMA out.
"""

import sys

sys.path.insert(0, "/opt/trn_rl_repo")

import numpy as np

import concourse.bass as bass  # noqa: F401
import concourse.tile as tile
from concourse import bacc, mybir
from concourse.masks import make_identity

F32 = mybir.dt.float32
F32R = mybir.dt.float32r

T_FULL = 2048
C = 2048          # model dim (contraction for projections)
D = 128           # head dim
HPC = 4           # heads per core
P = 128
CI = C // P       # 16 contraction tiles
SCALE = 1.0 / float(np.sqrt(D))


def _phase1(nc, tc, T, xt_v, wq_v, wk_v, wv_v, qt_sb, kt_sb, v_sb, ident):
    """Projections: qT, kT (strip matmuls), v (via vT + PE transpose)."""
    TS = T // 512
    with (
        tc.tile_pool(name="p1w", bufs=1) as p1w,
        tc.tile_pool(name="p1x", bufs=2) as p1x,
        tc.tile_pool(name="p1s", bufs=2) as p1s,
        tc.tile_pool(name="p1ps", bufs=4, space="PSUM") as p1ps,
        tc.tile_pool(name="p1tp", bufs=2, space="PSUM") as p1tp,
    ):
        wq_sb = p1w.tile([P, CI, HPC * D], F32R, tag="wq")
        nc.sync.dma_start(wq_sb[:], wq_v)
        wk_sb = p1w.tile([P, CI, D], F32R, tag="wk")
        nc.sync.dma_start(wk_sb[:], wk_v)
        wv_sb = p1w.tile([P, CI, D], F32R, tag="wv")
        nc.sync.dma_start(wv_sb[:], wv_v)

        for s in range(TS):
            sl = slice(s * 512, (s + 1) * 512)
            xt_sb = p1x.tile([P, CI, 512], F32R, tag="xt")
            nc.sync.dma_start(xt_sb[:], xt_v[:, :, sl])

            for e in range(HPC):   # qT strips per head
                ps = p1ps.tile([P, 512], F32, tag="proj_ps")
                for ci in range(CI):
                    nc.tensor.matmul(
                        ps[:], lhsT=wq_sb[:, ci, e * D:(e + 1) * D],
                        rhs=xt_sb[:, ci, :],
                        start=(ci == 0), stop=(ci == CI - 1))
                nc.vector.tensor_copy(out=qt_sb[:, e, sl], in_=ps[:])

            ps = p1ps.tile([P, 512], F32, tag="proj_ps")   # kT strip
            for ci in range(CI):
                nc.tensor.matmul(
                    ps[:], lhsT=wk_sb[:, ci, :],
                    rhs=xt_sb[:, ci, :],
                    start=(ci == 0), stop=(ci == CI - 1))
            nc.vector.tensor_copy(out=kt_sb[:, sl], in_=ps[:])

            ps = p1ps.tile([P, 512], F32, tag="proj_ps")   # vT strip
            for ci in range(CI):
                nc.tensor.matmul(
                    ps[:], lhsT=wv_sb[:, ci, :],
                    rhs=xt_sb[:, ci, :],
                    start=(ci == 0), stop=(ci == CI - 1))
            vt_sb = p1s.tile([P, 512], F32, tag="vt")
            nc.vector.tensor_copy(out=vt_sb[:], in_=ps[:])
            for k in range(4):     # PE transpose -> v natural [t, d]
                tp = p1tp.tile([P, P], F32, tag="tp_ps")
                nc.tensor.transpose(tp[:], vt_sb[:, k * P:(k + 1) * P], ident[:])
                nc.vector.tensor_copy(out=v_sb[:, s * 4 + k, :], in_=tp[:])


def _phase2(nc, tc, T, qt_sb, kt_sb, v_sb, yt_sb, mask_sb, ones_col, ones_row):
    """Attention: S^T blocks, exp, causal mask, denom + yT accumulation."""
    TS = T // 512
    with (
        tc.tile_pool(name="p2p", bufs=3) as p2p,
        tc.tile_pool(name="p2n", bufs=2) as p2n,
        tc.tile_pool(name="p2s", bufs=2, space="PSUM") as p2s,
        tc.tile_pool(name="p2y", bufs=2, space="PSUM") as p2y,
        tc.tile_pool(name="p2d", bufs=1, space="PSUM") as p2d,
        tc.tile_pool(name="p2b", bufs=1, space="PSUM") as p2b,
    ):
        for h in range(HPC):
            for s in range(TS):
                sl = slice(s * 512, (s + 1) * 512)
                nblk = 4 * s + 4        # causal: tk tiles j = 0..nblk-1
                yt_ps = p2y.tile([P, 512], F32, tag="yt_ps")
                dn_ps = p2d.tile([1, 512], F32, tag="dn_ps")
                for jp in range(0, nblk, 2):
                    s_ps = p2s.tile([P, 2, 512], F32, tag="s_ps")
                    for u in range(2):
                        j = jp + u
                        nc.tensor.matmul(
                            s_ps[:, u, :],
                            lhsT=kt_sb[:, j * P:(j + 1) * P],
                            rhs=qt_sb[:, h, sl],
                            start=True, stop=True)
                    p_sb = p2p.tile([P, 2, 512], F32R, tag="p_sb")
                    nc.scalar.activation(
                        p_sb[:], s_ps[:],
                        mybir.ActivationFunctionType.Exp, scale=SCALE)
                    for u in range(2):
                        b = jp + u - 4 * s
                        if b >= 0:      # diagonal block: causal mask
                            nc.vector.tensor_mul(
                                out=p_sb[:, u, :], in0=p_sb[:, u, :],
                                in1=mask_sb[:, b, :])
                    for u in range(2):
                        j = jp + u
                        nc.tensor.matmul(
                            yt_ps[:], lhsT=v_sb[:, j, :],
                            rhs=p_sb[:, u, :],
                            start=(j == 0), stop=(j == nblk - 1))
                        nc.tensor.matmul(
                            dn_ps[:], lhsT=ones_col,
                            rhs=p_sb[:, u, :],
                            start=(j == 0), stop=(j == nblk - 1))
                # normalize: PE broadcast of denom row, DVE reciprocal, mul
                dnrow_sb = p2n.tile([1, 512], F32R, tag="dnrow")
                nc.scalar.copy(out=dnrow_sb[:], in_=dn_ps[:])
                bc_ps = p2b.tile([P, 512], F32, tag="bc_ps")
                nc.tensor.matmul(
                    bc_ps[:], lhsT=ones_row,
                    rhs=dnrow_sb[:], start=True, stop=True)
                drecip = p2n.tile([P, 512], F32, tag="drecip")
                nc.vector.reciprocal_approx_fast(out=drecip[:], in_=bc_ps[:])
                nc.vector.tensor_mul(
                    out=yt_sb[:, h, sl], in0=yt_ps[:], in1=drecip[:])


def _phase3(nc, tc, T, yt_sb, wc_sb, out_v):
    """c_proj: out[t, o] over 4 head-dim contraction blocks."""
    TT = T // P
    with (
        tc.tile_pool(name="p3o", bufs=2) as p3o,
        tc.tile_pool(name="p3ps", bufs=2, space="PSUM") as p3ps,
    ):
        for tt in range(TT):
            out_sb = p3o.tile([P, 4, 512], F32, tag="out_sb")
            for os_ in range(4):
                ps = p3ps.tile([P, 512], F32, tag="cp_ps")
                for hh in range(HPC):
                    nc.tensor.matmul(
                        ps[:],
                        lhsT=yt_sb[:, hh, tt * P:(tt + 1) * P],
                        rhs=wc_sb[:, hh, os_ * 512:(os_ + 1) * 512],
                        start=(hh == 0), stop=(hh == HPC - 1))
                nc.vector.tensor_copy(out=out_sb[:, os_, :], in_=ps[:])
            nc.sync.dma_start(out_v[:, tt], out_sb[:])


def build_nc(T=T_FULL):
    """Build and compile the per-core Bass module. T: multiple of 512."""
    assert T % 512 == 0
    nc = bacc.Bacc("TRN2", target_bir_lowering=False, debug=False,
                   enable_asserts=True, num_devices=8)

    xt_d = nc.dram_tensor("xt", [C, T], F32R, kind="ExternalInput").ap()
    wq_d = nc.dram_tensor("wq", [C, HPC * D], F32R, kind="ExternalInput").ap()
    wk_d = nc.dram_tensor("wk", [C, D], F32R, kind="ExternalInput").ap()
    wv_d = nc.dram_tensor("wv", [C, D], F32R, kind="ExternalInput").ap()
    wc_d = nc.dram_tensor("wc", [HPC * D, C], F32R, kind="ExternalInput").ap()
    mask_d = nc.dram_tensor("mask", [4, P, 512], F32R, kind="ExternalInput").ap()
    ones_d = nc.dram_tensor("ones", [P, P], F32R, kind="ExternalInput").ap()
    out_d = nc.dram_tensor("out", [T, C], F32, kind="ExternalOutput").ap()

    xt_v = xt_d.rearrange("(ci p) t -> p ci t", p=P)
    wq_v = wq_d.rearrange("(ci p) e -> p ci e", p=P)
    wk_v = wk_d.rearrange("(ci p) d -> p ci d", p=P)
    wv_v = wv_d.rearrange("(ci p) d -> p ci d", p=P)
    wc_v = wc_d.rearrange("(hh p) o -> p hh o", p=P)
    mask_v = mask_d.rearrange("b p c -> p b c")
    out_v = out_d.rearrange("(tt p) (os o) -> p tt os o", p=P, o=512)

    with tile.TileContext(nc) as tc:
        with (
            tc.tile_pool(name="consts", bufs=1) as consts,
            tc.tile_pool(name="persist", bufs=1) as persist,
        ):
            mask_sb = consts.tile([P, 4, 512], F32R, tag="mask")
            nc.sync.dma_start(mask_sb[:], mask_v)
            ones_sb = consts.tile([P, P], F32R, tag="ones")
            nc.sync.dma_start(ones_sb[:], ones_d)
            ones_col = ones_sb[:, 0:1]
            ones_row = ones_sb[0:1, :]
            ident = consts.tile([P, P], F32, tag="ident")
            make_identity(nc, ident[:])

            qt_sb = persist.tile([P, HPC, T], F32R, tag="qt")   # [d, head, t]
            kt_sb = persist.tile([P, T], F32R, tag="kt")        # [d, t]
            v_sb = persist.tile([P, T // P, D], F32R, tag="v")  # [t, tt, d]

            _phase1(nc, tc, T, xt_v, wq_v, wk_v, wv_v,
                    qt_sb, kt_sb, v_sb, ident)

            # wc/yt live in a pool opened after phase 1 frees its SBUF
            with tc.tile_pool(name="late", bufs=1) as late:
                wc_sb = late.tile([P, HPC, C], F32R, tag="wc")
                nc.sync.dma_start(wc_sb[:], wc_v)
                yt_sb = late.tile([P, HPC, T], F32R, tag="yt")  # [d, head, t]

                _phase2(nc, tc, T, qt_sb, kt_sb, v_sb, yt_sb,
                        mask_sb, ones_col, ones_row)
                _phase3(nc, tc, T, yt_sb, wc_sb, out_v)

    nc.compile()
    return nc


def make_masks():
    r = np.arange(P)[:, None]
    c = np.arange(512)[None, :]
    return np.ascontiguousarray(
        np.stack([(c >= 128 * b + r) for b in range(4)]).astype(np.float32))


def make_in_maps(x, Wq, Wkv, Wc):
    masks = make_masks()
    in_maps = []
    for core in range(8):
        b, g = core // 4, core % 4
        in_maps.append({
            "xt": np.ascontiguousarray(np.asarray(x[b]).T),
            "wq": np.ascontiguousarray(np.asarray(Wq[:, 512 * g:512 * (g + 1)])),
            "wk": np.ascontiguousarray(np.asarray(Wkv[:, 128 * g:128 * (g + 1)])),
            "wv": np.ascontiguousarray(
                np.asarray(Wkv[:, 512 + 128 * g:512 + 128 * (g + 1)])),
            "wc": np.ascontiguousarray(np.asarray(Wc[512 * g:512 * (g + 1), :])),
            "mask": masks,
            "ones": np.ones((P, P), np.float32),
        })
    return in_maps


_NC_CACHE = {}


def _get_nc():
    if "nc" not in _NC_CACHE:
        _NC_CACHE["nc"] = build_nc()
    return _NC_CACHE["nc"]


def run(x, Wq, Wkv, Wc, trace=False, **kwargs):
    from concourse.bass_utils import run_bass_kernel_spmd
    nc = _get_nc()
    in_maps = make_in_maps(x, Wq, Wkv, Wc)
    res = run_bass_kernel_spmd(nc, in_maps, list(range(8)), trace=trace, **kwargs)
    B, T, C_ = x.shape
    out = np.empty((B, T, C_), np.float32)
    for b in range(B):
        acc = res.results[4 * b]["out"].astype(np.float32)
        for g in range(1, 4):
            acc = acc + res.results[4 * b + g]["out"]
        out[b] = acc
    return out, res


def kernel(x, Wq, Wkv, Wc):
    out, _ = run(x, Wq, Wkv, Wc, trace=False)
    return out
